# revision 29
# baseline (speedup 1.0000x reference)
"""Trainium2 Bass kernel for nn_AdaptiveMobiusLayer.

Strategy (pure data parallel over tokens, 8 NeuronCores):
  - Flatten x [4, 4096, 1024] -> [16384, 1024] tokens; core c takes 2048
    consecutive tokens (= batch b = c//2, seq half c%2).
  - Host transposes each shard to [1024 feats, 2048 tokens] so every matmul
    keeps features on partitions (weights are natural [K, M] lhsT stationary
    operands, activations are the moving operand; no on-device transposes).
  - The seq-mean for the global context needs the partner core's partial sum:
    one tiny pairwise AllReduce ([128, 8] f32) overlapped with cycle-1 compute.
  - MLP matmuls run in bf16 (fp32 accumulation in PSUM); `out` stays fp32 in
    SBUF across all 3 cycles.  Measured emulation rel_err ~9e-5.
"""

import sys

sys.path.insert(0, "/opt/trn_rl_repo")

import numpy as np

B, S, DIM = 4, 4096, 1024
NCORES = 8
TOK = B * S // NCORES  # 2048 tokens per core
CHUNK = 512
NCHUNK = TOK // CHUNK  # 4
NUM_CYCLES = 3
BASE_COUPLING = 0.1

# feature-quarter twist:  out_new[t] = out[t] + sign[t] * c * out[(t+4) % 8]
# tiles 0..7 are 128-feature slabs; quarters = [t0 t1 | t2 t3 | t4 t5 | t6 t7]
TWIST_SIGN = [+1, +1, -1, -1, -1, -1, +1, +1]

_CACHE = {}


def _build_graph():
    import concourse.bass as bass
    import concourse.bacc as bacc
    import concourse.tile as tile
    import concourse.mybir as mybir

    f32 = mybir.dt.float32
    bf16 = mybir.dt.bfloat16
    AF = mybir.ActivationFunctionType
    ALU = mybir.AluOpType
    AX = mybir.AxisListType

    nc = bacc.Bacc(
        "TRN2", target_bir_lowering=False, debug=False, num_devices=NCORES
    )

    # ---- DRAM parameters (per-core shard; layouts prepared on host) ----
    x_d = nc.declare_dram_parameter("x", [DIM, TOK], f32, isOutput=False)
    f8 = mybir.dt.float8e4
    DR = mybir.MatmulPerfMode.DoubleRow
    # coupling-net weights in fp8 (DoubleRow 2x matmul mode), packed in SBUF
    # tile layout, fo-major: row fo*128+p, col s*128+j == w[s*128+p, fo*128+j]
    w1_d = nc.declare_dram_parameter("cn_w1", [8 * 128, DIM], f8, isOutput=False)
    w2_d = nc.declare_dram_parameter("cn_w2", [4 * 128, DIM], f8, isOutput=False)
    w3_d = nc.declare_dram_parameter("cn_w3", [2 * 128, 512], f8, isOutput=False)
    w4_d = nc.declare_dram_parameter("cn_w4", [128, 2, 1], f8, isOutput=False)
    # all biases + scalars packed into one small tensor (single DMA):
    # cols 0-7 b1, 8-11 b2, 12-13 b3, 14-17 gb1, 18-19 gb2;
    # partition-0 scalars: [0,20]=b4 [0,21]=gb3 [0,22]=adaptive_range
    cst_d = nc.declare_dram_parameter("consts", [128, 23], f32, isOutput=False)
    gw1_d = nc.declare_dram_parameter("gc_w1", [DIM, 512], bf16, isOutput=False)
    gw2_d = nc.declare_dram_parameter("gc_w2", [512, 256], bf16, isOutput=False)
    gw3_d = nc.declare_dram_parameter("gc_w3", [256, 1], bf16, isOutput=False)
    out_d = nc.declare_dram_parameter("out", [DIM, TOK], f32, isOutput=True)

    with tile.TileContext(nc) as tc:
        with (
            tc.tile_pool(name="const", bufs=1) as const,
            tc.tile_pool(name="xres", bufs=1) as xres,
            tc.tile_pool(name="work", bufs=2) as work,
            tc.tile_pool(name="psm", bufs=6, space="PSUM") as psm,
            tc.tile_pool(name="psx", bufs=2, space="PSUM") as psx,
            tc.tile_pool(name="dram", bufs=1, space="DRAM") as dram,
        ):
            # ---------------- weight / bias loads ----------------
            def load_w(dparam, kin, nout, tagp):
                tiles = []
                for k in range(kin // 128):
                    t = const.tile([128, nout], bf16, tag=f"{tagp}_{k}")
                    dma_rr(t[:], dparam[k * 128:(k + 1) * 128, :])
                    tiles.append(t)
                return tiles

            # dma_start ISSUE costs ~0.6us on a sequencer; round-robin the
            # loads across four otherwise-idle engine sequencers so issue
            # isn't the startup critical path.
            # dma_start ISSUE costs ~0.6us on a sequencer, serially.  Only
            # sync has the fast HWDGE path for bulk; the ACT sequencer is idle
            # for the first ~15us, so it issues the small early const/weight
            # DMAs, letting sync start on x immediately.
            early = [0]

            def dma_rr(out, in_):
                if early[0] > 0:
                    early[0] -= 1
                    nc.scalar.dma_start(out=out, in_=in_)
                else:
                    nc.sync.dma_start(out=out, in_=in_)

            def dma_out_rr(out, in_):
                nc.sync.dma_start(out=out, in_=in_)

            # bias/constant tiles first (a late bias DMA gates every GELU on
            # the in-order ACT queue); one packed DMA.
            early[0] = 9  # cst + the 8 w1f tiles go on the ACT sequencer
            cst = const.tile([128, 23], f32, tag="cst")
            dma_rr(cst[:], cst_d[:, :])
            b1 = cst[:, 0:8]
            b2 = cst[:, 8:12]
            b3 = cst[:, 12:14]
            gb1 = cst[:, 14:18]
            gb2 = cst[:, 18:20]
            b4 = cst[0:1, 20:21]
            gb3 = cst[0:1, 21:22]
            ar = cst[0:1, 22:23]
            ones = const.tile([1, 128], bf16, tag="ones")
            nc.vector.memset(ones[:], 1.0)

            def load_w_fo(dparam, kin, nfo, tagp):
                """[128, nk, 128] fp8 tiles (k = s*128 + p), one per fo."""
                tiles = []
                nk = kin // 128
                for fo in range(nfo):
                    t = const.tile([128, nk, 128], f8, tag=f"{tagp}_{fo}")
                    dma_rr(t[:], dparam[fo * 128:(fo + 1) * 128, :].rearrange(
                        "p (s j) -> p s j", s=nk))
                    tiles.append(t)
                return tiles

            w1f = load_w_fo(w1_d, DIM, 8, "w1f")

            out_f32 = [[None] * NCHUNK for _ in range(8)]

            def load_x_chunk(c):
                for t in range(8):
                    tl = xres.tile([128, CHUNK], f32, tag=f"o_{t}_{c}")
                    dma_rr(
                        tl[:],
                        x_d[t * 128:(t + 1) * 128, c * CHUNK:(c + 1) * CHUNK],
                    )
                    out_f32[t][c] = tl

            load_x_chunk(0)
            load_x_chunk(1)
            w2f = load_w_fo(w2_d, DIM, 4, "w2f")
            w3f = load_w_fo(w3_d, 512, 2, "w3f")
            w4f = const.tile([128, 2, 1], f8, tag="w4f")
            dma_rr(w4f[:], w4_d[:, :, :])
            load_x_chunk(2)
            load_x_chunk(3)
            gw1 = load_w(gw1_d, DIM, 512, "gw1")
            gw2 = load_w(gw2_d, 512, 256, "gw2")
            gw3 = load_w(gw3_d, 256, 1, "gw3")

            # ---------------- global-context partial sums + AllReduce ----------------
            # per-(tile, chunk) partial sums; emitted per-chunk below so the
            # in-order DVE queue never blocks waiting for later x chunks.
            red = const.tile([128, 8, NCHUNK], f32, tag="gred")

            def reduce_chunk(c):
                for t in range(8):
                    nc.vector.tensor_reduce(
                        red[:, t, c:c + 1], out_f32[t][c][:], axis=AX.X, op=ALU.add
                    )

            gs = const.tile([128, 8], f32, tag="gs")

            def finish_gsum():
                for t in range(8):
                    nc.vector.tensor_reduce(
                        gs[:, t:t + 1], red[:, t, :], axis=AX.X, op=ALU.add
                    )

            cc_in = dram.tile([128, 8], f32, tag="cc_in")
            cc_out = dram.tile([128, 8], f32, tag="cc_out")
            gmean_f = const.tile([128, 8], f32, tag="gmean_f")
            gmean = const.tile([128, 8], bf16, tag="gmean")

            def do_collective():
                nc.sync.dma_start(out=cc_in[:], in_=gs[:])
                nc.gpsimd.collective_compute(
                    "AllReduce",
                    ALU.add,
                    ins=[cc_in.opt()],
                    outs=[cc_out.opt()],
                    replica_groups=[[0, 1], [2, 3], [4, 5], [6, 7]],
                )
                nc.sync.dma_start(out=gmean_f[:], in_=cc_out[:])
                nc.vector.tensor_copy(gmean[:], gmean_f[:])

            # ---------------- global net (emitted via closure; see cycle 0) -------
            gc_state = {}

            gc_tiles = {}

            def gc_stage1():
                # all 4 output-tile groups accumulate into one PSUM bank
                # (disjoint columns) -> a single GELU epilogue
                ps = psx.tile([128, 4], f32, tag="aux")
                for fo in range(4):
                    for k in range(8):
                        nc.tensor.matmul(
                            ps[:, fo:fo + 1], gw1[k][:, fo * 128:(fo + 1) * 128],
                            gmean[:, k:k + 1], start=(k == 0), stop=(k == 7),
                        )
                # psum holds gc_w1.T @ sum(x); fold the 1/S mean + bias on DVE
                # (activation bias APs must be [P,1]; gb1 varies per column)
                z1 = work.tile([128, 4], f32, tag="z1")
                nc.vector.scalar_tensor_tensor(
                    z1[:], ps[:], 1.0 / S, gb1, ALU.mult, ALU.add
                )
                g1 = work.tile([128, 4], bf16, tag="g1")
                nc.scalar.activation(g1[:], z1[:], AF.Gelu)
                gc_tiles["g1"] = g1

            def gc_stage2():
                g1 = gc_tiles["g1"]
                ps = psx.tile([128, 2], f32, tag="aux")
                for fo in range(2):
                    for k in range(4):
                        nc.tensor.matmul(
                            ps[:, fo:fo + 1], gw2[k][:, fo * 128:(fo + 1) * 128],
                            g1[:, k:k + 1], start=(k == 0), stop=(k == 3),
                        )
                z2 = work.tile([128, 2], f32, tag="z2")
                nc.vector.tensor_add(z2[:], ps[:], gb2)
                g2 = work.tile([128, 2], bf16, tag="g2")
                nc.scalar.activation(g2[:], z2[:], AF.Gelu)
                gc_tiles["g2"] = g2

            def gc_stage3():
                g2 = gc_tiles["g2"]
                ps = psx.tile([1, 1], f32, tag="aux")
                for k in range(2):
                    nc.tensor.matmul(
                        ps[:], gw3[k][:, 0:1], g2[:, k:k + 1],
                        start=(k == 0), stop=(k == 1)
                    )
                gf = const.tile([1, 1], f32, tag="gf")
                nc.scalar.activation(gf[:], ps[:], AF.Sigmoid, bias=gb3)

                # coupling = 0.1 + ar*(2*(0.7*gf + 0.3*tf) - 1)
                #          = [0.1 + ar*(1.4*gf - 1)] + (0.6*ar) * tf = c0 + cmul*tf
                cmul = const.tile([1, 1], f32, tag="cmul")
                nc.vector.tensor_scalar(cmul[:], ar, 0.6, None, ALU.mult)
                tmp0 = const.tile([1, 1], f32, tag="tmp0")
                nc.vector.tensor_scalar(tmp0[:], gf[:], 1.4, -1.0, ALU.mult, ALU.add)
                c0 = const.tile([1, 1], f32, tag="c0")
                nc.vector.tensor_tensor(tmp0[:], ar, tmp0[:], ALU.mult)
                nc.vector.tensor_scalar(c0[:], tmp0[:], BASE_COUPLING, None, ALU.add)
                gc_state["cmul"] = cmul
                gc_state["c0"] = c0

            # ---------------- per-chunk building blocks ----------------
            pending_xb = [None] * NCHUNK

            def conv_chunk(c, tile=None):
                tb = tile if tile is not None else work.tile(
                    [128, 8, CHUNK], f8, tag="xb")
                for t in range(8):
                    nc.vector.tensor_copy(tb[:, t, :], out_f32[t][c][:])
                return tb

            def mlp_chunk(c, hooks=()):
                """coupling-net MLP on chunk c of `out`; returns the tf tile.

                hooks: up to 3 closures emitted after L1/L2/L3 — lets the tiny
                serial gc-net chain ride the PE queue where each link's ACT
                dependency has had a full layer's worth of matmuls to finish.
                """
                hooks = list(hooks) + [None] * 3
                if pending_xb[c] is not None:
                    xb = pending_xb[c]
                    pending_xb[c] = None
                else:
                    xb = conv_chunk(c)
                h1 = work.tile([128, 8, CHUNK], f8, tag="h1")
                for fo in range(8):
                    ps1 = psm.tile([128, CHUNK], f32, tag="mm")
                    for s in range(4):
                        nc.tensor.matmul(
                            ps1[:], w1f[fo][:, 2 * s:2 * s + 2, :],
                            xb[:, 2 * s:2 * s + 2, :],
                            start=(s == 0), stop=(s == 3), perf_mode=DR,
                        )
                    nc.scalar.activation(
                        h1[:, fo, :], ps1[:], AF.Gelu, bias=b1[:, fo:fo + 1])
                if hooks[0]:
                    hooks[0]()
                h2 = work.tile([128, 4, CHUNK], f8, tag="h2")
                for fo in range(4):
                    ps2 = psm.tile([128, CHUNK], f32, tag="mm")
                    for s in range(4):
                        nc.tensor.matmul(
                            ps2[:], w2f[fo][:, 2 * s:2 * s + 2, :],
                            h1[:, 2 * s:2 * s + 2, :],
                            start=(s == 0), stop=(s == 3), perf_mode=DR,
                        )
                    nc.scalar.activation(
                        h2[:, fo, :], ps2[:], AF.Gelu, bias=b2[:, fo:fo + 1])
                if hooks[1]:
                    hooks[1]()
                h3 = work.tile([128, 2, CHUNK], f8, tag="h3")
                for fo in range(2):
                    ps3 = psm.tile([128, CHUNK], f32, tag="mm")
                    for s in range(2):
                        nc.tensor.matmul(
                            ps3[:], w3f[fo][:, 2 * s:2 * s + 2, :],
                            h2[:, 2 * s:2 * s + 2, :],
                            start=(s == 0), stop=(s == 1), perf_mode=DR,
                        )
                    nc.scalar.activation(
                        h3[:, fo, :], ps3[:], AF.Gelu, bias=b3[:, fo:fo + 1])
                if hooks[2]:
                    hooks[2]()
                ps4 = psx.tile([1, CHUNK], f32, tag="aux")
                for s in range(2):
                    nc.tensor.matmul(
                        ps4[:], w4f[:, s, :], h3[:, s, :],
                        start=(s == 0), stop=(s == 1),
                    )
                tf = work.tile([1, CHUNK], f32, tag=f"tf_{c}")
                nc.scalar.activation(tf[:], ps4[:], AF.Sigmoid, bias=b4)
                return tf

            def update_chunk(c, tf, last, next_conv=False):
                """coupling + twist update (in place) on chunk c; DMA out if last."""
                coup = work.tile([1, CHUNK], f32, tag="coup")
                nc.vector.tensor_scalar(
                    coup[:], tf[:], gc_state["cmul"][:], gc_state["c0"][:],
                    ALU.mult, ALU.add,
                )
                # broadcast coupling across partitions via two bf16 K=1
                # matmuls (hi + residual lo: ~fp32 precision at bf16 speed)
                c_hi = work.tile([1, CHUNK], bf16, tag="c_hi")
                nc.vector.tensor_copy(c_hi[:], coup[:])
                c_rem = work.tile([1, CHUNK], f32, tag="c_rem")
                nc.vector.tensor_sub(c_rem[:], coup[:], c_hi[:])
                c_lo = work.tile([1, CHUNK], bf16, tag="c_lo")
                nc.vector.tensor_copy(c_lo[:], c_rem[:])
                psb = psx.tile([128, CHUNK], f32, tag="aux")
                nc.tensor.matmul(psb[:], ones[:], c_hi[:], start=True, stop=False)
                nc.tensor.matmul(psb[:], ones[:], c_lo[:], start=False, stop=True)
                cb = work.tile([128, CHUNK], f32, tag="cb")
                nc.vector.tensor_copy(cb[:], psb[:])
                # twist update: pairs (t, t+4); all reads precede writes
                xb_next = None
                for p in range(4):
                    t, u = p, p + 4
                    # pair 3 runs on gpsimd (otherwise idle) to shed DVE load
                    eng = nc.gpsimd if p == 3 else nc.vector
                    tmpa = work.tile([128, CHUNK], f32, tag="twa")
                    tmpb = work.tile([128, CHUNK], f32, tag="twb")
                    eng.tensor_mul(tmpa[:], out_f32[u][c][:], cb[:])
                    eng.tensor_mul(tmpb[:], out_f32[t][c][:], cb[:])
                    if TWIST_SIGN[t] > 0:
                        eng.tensor_add(out_f32[t][c][:], out_f32[t][c][:], tmpa[:])
                    else:
                        eng.tensor_sub(out_f32[t][c][:], out_f32[t][c][:], tmpa[:])
                    if TWIST_SIGN[u] > 0:
                        eng.tensor_add(out_f32[u][c][:], out_f32[u][c][:], tmpb[:])
                    else:
                        eng.tensor_sub(out_f32[u][c][:], out_f32[u][c][:], tmpb[:])
                    if last:
                        for tt in (t, u):
                            dma_out_rr(
                                out_d[tt * 128:(tt + 1) * 128,
                                      c * CHUNK:(c + 1) * CHUNK],
                                out_f32[tt][c][:],
                            )
                    elif next_conv:
                        # next cycle's fp8 conversion for this pair; ACT has
                        # slack (DVE is the critical engine), Copy casts f32->f8
                        if xb_next is None:
                            xb_next = work.tile([128, 8, CHUNK], f8, tag="xb")
                        nc.scalar.activation(
                            xb_next[:, t, :], out_f32[t][c][:], AF.Copy)
                        nc.scalar.activation(
                            xb_next[:, u, :], out_f32[u][c][:], AF.Copy)
                if next_conv and not last:
                    pending_xb[c] = xb_next

            # ---------------- main cycles ----------------
            # Cycle 0: all chunk MLPs first, THEN the gc-net (whose collective
            # input is only ready once all of x has landed), then the
            # couplings/updates.  Putting the gc-net matmuls early would park
            # them at the head of the in-order PE queue, stalling it ~30us.
            tf0 = []
            for c in range(NCHUNK):
                tf0.append(mlp_chunk(c))
                reduce_chunk(c)
                if c == NCHUNK - 1:
                    finish_gsum()
                    do_collective()
            gc_stage1()
            gc_stage2()
            gc_stage3()
            for c in range(NCHUNK):
                update_chunk(c, tf0[c], last=False, next_conv=True)
            for cyc in range(1, NUM_CYCLES):
                last = cyc == NUM_CYCLES - 1
                for c in range(NCHUNK):
                    tf = mlp_chunk(c)
                    update_chunk(c, tf, last, next_conv=not last)

    nc.compile()
    return nc


def _get_graph():
    if "nc" not in _CACHE:
        _CACHE["nc"] = _build_graph()
    return _CACHE["nc"]


def _pack_consts(inputs):
    cst = np.zeros((128, 23), np.float32)
    cst[:, 0:8] = np.asarray(inputs["cn_b1"], np.float32).reshape(8, 128).T
    cst[:, 8:12] = np.asarray(inputs["cn_b2"], np.float32).reshape(4, 128).T
    cst[:, 12:14] = np.asarray(inputs["cn_b3"], np.float32).reshape(2, 128).T
    cst[:, 14:18] = np.asarray(inputs["gc_b1"], np.float32).reshape(4, 128).T
    cst[:, 18:20] = np.asarray(inputs["gc_b2"], np.float32).reshape(2, 128).T
    cst[0, 20] = np.asarray(inputs["cn_b4"], np.float32).reshape(())
    cst[0, 21] = np.asarray(inputs["gc_b3"], np.float32).reshape(())
    cst[0, 22] = np.asarray(inputs["adaptive_range"], np.float32).reshape(())
    return cst


def _make_in_maps(inputs):
    import ml_dtypes

    bf = ml_dtypes.bfloat16
    f8 = ml_dtypes.float8_e4m3
    x = np.ascontiguousarray(inputs["x"], dtype=np.float32)
    xs = x.reshape(NCORES, TOK, DIM).transpose(0, 2, 1)  # [8, 1024, 2048]

    shared = {
        "cn_w1": np.ascontiguousarray(
            np.asarray(inputs["cn_w1"]).reshape(8, 128, 8, 128)
            .transpose(2, 1, 0, 3).reshape(8 * 128, DIM), dtype=f8),
        "cn_w2": np.ascontiguousarray(
            np.asarray(inputs["cn_w2"]).reshape(8, 128, 4, 128)
            .transpose(2, 1, 0, 3).reshape(4 * 128, DIM), dtype=f8),
        "cn_w3": np.ascontiguousarray(
            np.asarray(inputs["cn_w3"]).reshape(4, 128, 2, 128)
            .transpose(2, 1, 0, 3).reshape(2 * 128, 512), dtype=f8),
        "cn_w4": np.ascontiguousarray(
            np.asarray(inputs["cn_w4"]).reshape(2, 128).T.reshape(128, 2, 1),
            dtype=f8),
        "gc_w1": np.ascontiguousarray(inputs["gc_w1"], dtype=bf),
        "gc_w2": np.ascontiguousarray(inputs["gc_w2"], dtype=bf),
        "gc_w3": np.ascontiguousarray(inputs["gc_w3"].reshape(256, 1), dtype=bf),
        "consts": _pack_consts(inputs),
    }
    in_maps = []
    for c in range(NCORES):
        m = dict(shared)
        m["x"] = np.ascontiguousarray(xs[c])
        in_maps.append(m)
    return in_maps


def _run(inputs, trace=False):
    from concourse.bass_utils import run_bass_kernel_spmd

    nc = _get_graph()
    in_maps = _make_in_maps(inputs)
    res = run_bass_kernel_spmd(
        nc, in_maps, core_ids=list(range(NCORES)), trace=trace
    )
    outs = np.stack(
        [np.asarray(res.results[c]["out"]).T for c in range(NCORES)], axis=0
    )  # [8, 2048, 1024]
    full = outs.reshape(B, S, DIM).astype(np.float32)
    return full, res


def kernel(**inputs) -> np.ndarray:
    out, _ = _run(inputs, trace=False)
    return out
